# revision 11
# baseline (speedup 1.0000x reference)
"""MHSA block (b=8, c=256, h=w=32, nh=8) on 8 Trainium2 cores.

Sharding: pure data parallel -- one batch element per NeuronCore, no
collectives.  Per-core algorithm (X = x[b] as (C=256, L=1024)):

  QK   = Wqk @ X (+bqk via ACT bias on psum->sbuf copy)   (512, L)
  V^T  = X^T @ Wv^T + bv  (head-padded, ones col h*33+32) (L, 264)
  S^T_h = K_h^T Q_h  (per head, j on partitions)          -> exp(scale*S^T) = P^T
  [O_h; l_h] = V_aug,h^T.T @ P^T_h   (PSUM accumulate)    row 32/96 = softmax denom
  O_norm = O * replicate(1/l)        (PE K=2 matmul replication)
  out  = (x + bproj) + Wproj @ O_norm   (bproj folded into residual host-side)

All matmul operands are bf16 (hw accumulates fp32 in PSUM); exp on ScalarE is
the throughput floor (~8.4M elems/core, ~70us).  Two heads share each S^T PE
pass via 32-row groups.  Small filler matmuls pad PE idle gaps so the HAM
clock-gate keeps the PE at 2.4 GHz through the exp-gated phase.
Normalization + projection are pipelined per 512-column half-stream.
"""

import sys
import os

sys.path.insert(0, "/opt/trn_rl_repo")

from contextlib import ExitStack

import numpy as np

NH, DH, C, L = 8, 32, 256, 1024
B = 8
SCALE = DH ** -0.5
N_CORES = 8
FILLER_N = 256
FILLERS_PER_ITER = 1


_CACHE = {}


def _build_nc():
    import concourse.tile as tile
    from concourse import bacc, mybir

    f32 = mybir.dt.float32
    bf16 = mybir.dt.bfloat16
    Exp = mybir.ActivationFunctionType.Exp
    Identity = mybir.ActivationFunctionType.Identity

    nc = bacc.Bacc("TRN2", target_bir_lowering=False, debug=False)

    x_d = nc.dram_tensor("x", [C, L], bf16, kind="ExternalInput").ap()
    xf_d = nc.dram_tensor("xf", [C, L], f32, kind="ExternalInput").ap()
    wqkT_d = nc.dram_tensor("wqkT", [C, 512], bf16, kind="ExternalInput").ap()
    bqkc_d = nc.dram_tensor("bqkc", [128, 4], f32, kind="ExternalInput").ap()
    wvT_d = nc.dram_tensor("wvT", [C, 264], bf16, kind="ExternalInput").ap()
    bv_d = nc.dram_tensor("bv", [1, 264], bf16, kind="ExternalInput").ap()
    wpT_d = nc.dram_tensor("wpT", [512, 256], bf16, kind="ExternalInput").ap()
    e_d = nc.dram_tensor("ee", [2, 128], bf16, kind="ExternalInput").ap()
    ones_d = nc.dram_tensor("ones_in", [1, 512], bf16, kind="ExternalInput").ap()
    out_d = nc.dram_tensor("out", [C, L], f32, kind="ExternalOutput").ap()

    with tile.TileContext(nc) as tc, ExitStack() as ctx:
        persist = ctx.enter_context(tc.tile_pool(name="persist", bufs=1))
        ptpool = ctx.enter_context(tc.tile_pool(name="pt", bufs=3))
        onpool = ctx.enter_context(tc.tile_pool(name="on", bufs=2))
        smallp = ctx.enter_context(tc.tile_pool(name="small", bufs=2))
        stps = ctx.enter_context(tc.tile_pool(name="stps", bufs=2, space="PSUM"))
        pvps = ctx.enter_context(tc.tile_pool(name="pvps", bufs=1, space="PSUM"))
        wkps = ctx.enter_context(tc.tile_pool(name="wkps", bufs=1, space="PSUM"))
        filps = ctx.enter_context(tc.tile_pool(name="filps", bufs=1, space="PSUM"))

        # ---- constants / inputs to SBUF ----
        ones = persist.tile([1, 512], bf16, tag="ones", name="ones")
        nc.sync.dma_start(ones[:], ones_d[:])

        # warm the ACT exp table while the QKV phase runs
        warm = persist.tile([1, 8], f32, tag="warm", name="warm")
        nc.gpsimd.memset(warm[:], 0.0)
        nc.scalar.activation(warm[:], warm[:], Exp)

        x_sb = []
        for t in range(2):
            xt = persist.tile([128, L], bf16, tag=f"x{t}", name=f"x{t}")
            nc.sync.dma_start(xt[:], x_d[t * 128:(t + 1) * 128, :])
            x_sb.append(xt)

        wqk_sb = []
        for t in range(2):
            w = persist.tile([128, 512], bf16, tag=f"wqk{t}", name=f"wqk{t}")
            nc.sync.dma_start(w[:], wqkT_d[t * 128:(t + 1) * 128, :])
            wqk_sb.append(w)
        bqkc_sb = persist.tile([128, 4], f32, tag="bqkc", name="bqkc")
        nc.sync.dma_start(bqkc_sb[:], bqkc_d[:])

        wv_sb = []
        for t in range(2):
            w = persist.tile([128, 264], bf16, tag=f"wv{t}", name=f"wv{t}")
            nc.sync.dma_start(w[:], wvT_d[t * 128:(t + 1) * 128, :])
            wv_sb.append(w)
        bv_sb = persist.tile([1, 264], bf16, tag="bv", name="bv")
        nc.sync.dma_start(bv_sb[:], bv_d[:])

        wp_sb = []
        for p in range(4):
            w = persist.tile([128, 256], bf16, tag=f"wp{p}", name=f"wp{p}")
            nc.sync.dma_start(w[:], wpT_d[p * 128:(p + 1) * 128, :])
            wp_sb.append(w)

        e_sb = persist.tile([2, 128], bf16, tag="ee", name="ee")
        nc.sync.dma_start(e_sb[:], e_d[:])

        o_sb = []
        for p in range(4):
            o = persist.tile([128, L], f32, tag=f"o{p}", name=f"o{p}")
            nc.gpsimd.memset(o[:], 0.0)
            o_sb.append(o)

        acc = [persist.tile([128, L], f32, tag=f"acc{t}", name=f"acc{t}") for t in range(2)]

        xf_sb = []
        for t in range(2):
            xft = persist.tile([128, L], f32, tag=f"xf{t}", name=f"xf{t}")
            nc.sync.dma_start(xft[:], xf_d[t * 128:(t + 1) * 128, :])
            xf_sb.append(xft)

        # ---- QK gemm:  QK(512, L) = WqkT.T @ X; bqk added on the ACT copy ----
        qk_sb = [None] * 4

        def qk_chunk(mt):
            ps = stps.tile([128, L], f32, tag="st", name="qkps")
            for nh_ in range(2):
                o = ps[:, nh_ * 512:(nh_ + 1) * 512]
                for kt in range(2):
                    nc.tensor.matmul(
                        o,
                        lhsT=wqk_sb[kt][:, mt * 128:(mt + 1) * 128],
                        rhs=x_sb[kt][:, nh_ * 512:(nh_ + 1) * 512],
                        start=(kt == 0),
                        stop=(kt == 1),
                    )
            qk = persist.tile([128, L], bf16, tag=f"qk{mt}", name=f"qk{mt}")
            nc.scalar.activation(qk[:], ps[:], Identity, bias=bqkc_sb[:, mt:mt + 1])
            qk_sb[mt] = qk

        # ---- V^T gemm: VT(L, 264) = X.T @ WvT + bv  (head-padded cols) ----
        vt_sb = [None] * 8

        def vt_chunk(jt):
            ps = wkps.tile([128, 264], f32, tag="wk", name="vtps")
            for kt in range(2):
                nc.tensor.matmul(
                    ps[:],
                    lhsT=x_sb[kt][:, jt * 128:(jt + 1) * 128],
                    rhs=wv_sb[kt][:],
                    start=(kt == 0),
                    stop=False,
                )
            nc.tensor.matmul(
                ps[:],
                lhsT=ones[0:1, 0:128],
                rhs=bv_sb[0:1, :],
                start=False,
                stop=True,
            )
            vt = persist.tile([128, 264], bf16, tag=f"vt{jt}", name=f"vt{jt}")
            nc.vector.tensor_copy(vt[:], ps[:])
            vt_sb[jt] = vt

        qk_chunk(0)
        qk_chunk(2)

        def attention_stream(p, ih, interleave_vt=False):
            qt = qk_sb[p // 2]
            kt_ = qk_sb[2 + p // 2]
            oA = 64 * (p % 2)
            oB = oA + 32
            hA, hB = 2 * p, 2 * p + 1
            pvA = pvps.tile([33, 512], f32, tag="pvA", name="pvA")
            pvB = pvps.tile([33, 512], f32, tag="pvB", name="pvB")
            for jc in range(8):
                if interleave_vt:
                    vt_chunk(jc)
                st = stps.tile([128, L], f32, tag="st", name="st")
                nc.tensor.matmul(
                    st[:, 0:512],
                    lhsT=kt_[oA:oA + 32, jc * 128:(jc + 1) * 128],
                    rhs=qt[oA:oA + 32, ih * 512:(ih + 1) * 512],
                    start=True,
                    stop=True,
                    tile_position=(oA, 0),
                )
                nc.tensor.matmul(
                    st[:, 512:1024],
                    lhsT=kt_[oB:oB + 32, jc * 128:(jc + 1) * 128],
                    rhs=qt[oB:oB + 32, ih * 512:(ih + 1) * 512],
                    start=True,
                    stop=True,
                    tile_position=(oB, 0),
                )
                pt = ptpool.tile([128, L], bf16, tag="pt", name="pt")
                nc.scalar.activation(pt[:], st[:], Exp, scale=SCALE)
                nc.tensor.matmul(
                    pvA[:],
                    lhsT=vt_sb[jc][:, hA * 33:hA * 33 + 33],
                    rhs=pt[:, 0:512],
                    start=(jc == 0),
                    stop=(jc == 7),
                )
                nc.tensor.matmul(
                    pvB[:],
                    lhsT=vt_sb[jc][:, hB * 33:hB * 33 + 33],
                    rhs=pt[:, 512:1024],
                    start=(jc == 0),
                    stop=(jc == 7),
                )
                for _ in range(FILLERS_PER_ITER):
                    fl = filps.tile([1, FILLER_N], f32, tag="fil", name="fil")
                    nc.tensor.matmul(
                        fl[:],
                        lhsT=ones[0:1, 0:1],
                        rhs=qt[0:1, 0:FILLER_N],
                        start=True,
                        stop=True,
                    )
            cols = slice(ih * 512, (ih + 1) * 512)
            nc.vector.tensor_copy(o_sb[p][0:33, cols], pvA[:])
            nc.vector.tensor_copy(o_sb[p][64:97, cols], pvB[:])

            # ---- per-half-stream normalization + projection ----
            l_sb = smallp.tile([2, 512], f32, tag="l", name="l")
            nc.sync.dma_start(l_sb[0:1, :], o_sb[p][32:33, cols])
            nc.sync.dma_start(l_sb[1:2, :], o_sb[p][96:97, cols])
            rl32 = smallp.tile([2, 512], f32, tag="rl32", name="rl32")
            scr = smallp.tile([2, 512], f32, tag="rlscratch", name="rlscratch")
            nc.vector.reciprocal_approx_accurate(rl32[:], l_sb[:], scr[:])
            rl = smallp.tile([2, 512], bf16, tag="rl", name="rl")
            nc.vector.tensor_copy(rl[:], rl32[:])

            rp = wkps.tile([128, 512], f32, tag="wk", name="rp")
            nc.tensor.matmul(rp[:], lhsT=e_sb[:], rhs=rl[:], start=True, stop=True)
            on = onpool.tile([128, 512], bf16, tag="on", name="on")
            nc.vector.tensor_mul(on[:], o_sb[p][:, cols], rp[:])
            for mt2 in range(2):
                pj = wkps.tile([128, 512], f32, tag="wk", name="pj")
                nc.tensor.matmul(
                    pj[:],
                    lhsT=wp_sb[p][:, mt2 * 128:(mt2 + 1) * 128],
                    rhs=on[:],
                    start=True,
                    stop=True,
                )
                if p == 0:
                    nc.vector.tensor_add(acc[mt2][:, cols], xf_sb[mt2][:, cols], pj[:])
                else:
                    nc.vector.tensor_add(acc[mt2][:, cols], acc[mt2][:, cols], pj[:])
                if p == 3:
                    nc.sync.dma_start(out_d[mt2 * 128:(mt2 + 1) * 128, cols], acc[mt2][:, cols])

        attention_stream(0, 0, interleave_vt=True)
        qk_chunk(1)
        qk_chunk(3)
        attention_stream(0, 1)
        for p in range(1, 4):
            for ih in range(2):
                attention_stream(p, ih)

    nc.compile()
    return nc


def _get_nc():
    if "nc" not in _CACHE:
        _CACHE["nc"] = _build_nc()
    return _CACHE["nc"]


def _pack_weights(w_qkv, b_qkv, w_proj, b_proj):
    w_qkv = np.asarray(w_qkv, dtype=np.float32)
    b_qkv = np.asarray(b_qkv, dtype=np.float32)
    w_proj = np.asarray(w_proj, dtype=np.float32)
    b_proj = np.asarray(b_proj, dtype=np.float32)

    wqkT = np.ascontiguousarray(w_qkv[:512].T)                  # (256, 512)
    bqkc = np.ascontiguousarray(b_qkv[:512].reshape(4, 128).T)  # (128, 4)

    wvT = np.zeros((C, 264), dtype=np.float32)
    bv = np.zeros((1, 264), dtype=np.float32)
    for h in range(NH):
        wvT[:, h * 33:h * 33 + 32] = w_qkv[512 + h * 32:512 + (h + 1) * 32].T
        bv[0, h * 33:h * 33 + 32] = b_qkv[512 + h * 32:512 + (h + 1) * 32]
        bv[0, h * 33 + 32] = 1.0

    # o_sb row layout per pair tile p: head 2p at rows 0:32 (l at 32),
    # head 2p+1 at rows 64:96 (l at 96); all other rows zero.
    wpT = np.zeros((512, 256), dtype=np.float32)
    for p in range(4):
        wpT[p * 128 + 0:p * 128 + 32, :] = w_proj[:, (2 * p) * 32:(2 * p + 1) * 32].T
        wpT[p * 128 + 64:p * 128 + 96, :] = w_proj[:, (2 * p + 1) * 32:(2 * p + 2) * 32].T

    ee = np.zeros((2, 128), dtype=np.float32)
    ee[0, 0:32] = 1.0
    ee[1, 64:96] = 1.0
    ones_in = np.ones((1, 512), dtype=np.float32)
    return dict(wqkT=wqkT, wvT=wvT, bv=bv, wpT=wpT, ee=ee,
                ones_in=ones_in), bqkc, b_proj


def _bf16(a):
    import ml_dtypes

    return np.asarray(a).astype(ml_dtypes.bfloat16)


def _install_ntff_hook_module():
    """bass_utils wants antenv.axon_hooks for trace=True under axon; this
    image's antenv lacks it.  Inject an equivalent module into sys.modules."""
    if "antenv.axon_hooks" in sys.modules:
        return
    try:
        import antenv.axon_hooks  # noqa: F401

        return
    except ImportError:
        pass
    import contextlib
    import ctypes
    import types

    mod = types.ModuleType("antenv.axon_hooks")
    state = {"hook": None, "inited": False}

    def _default_hook():
        so_path = "/opt/axon/libaxon_pjrt.so"
        if not os.path.exists(so_path):
            return None
        lib = ctypes.CDLL(so_path)
        if not hasattr(lib, "axon_start_nrt_profile"):
            return None
        lib.axon_start_nrt_profile.argtypes = [
            ctypes.POINTER(ctypes.c_int64),
            ctypes.c_size_t,
        ]
        lib.axon_start_nrt_profile.restype = ctypes.c_int64
        lib.axon_stop_nrt_profile.argtypes = [ctypes.c_char_p]
        lib.axon_stop_nrt_profile.restype = ctypes.c_int64

        @contextlib.contextmanager
        def _hook(output_dir, device_ids):
            import jax

            jax.devices()
            if device_ids:
                ids = (ctypes.c_int64 * len(device_ids))(*device_ids)
                rc = lib.axon_start_nrt_profile(ids, len(device_ids))
            else:
                rc = lib.axon_start_nrt_profile(None, 0)
            if rc != 0:
                raise RuntimeError(f"axon_start_nrt_profile rc={rc}")
            try:
                yield
            finally:
                n = lib.axon_stop_nrt_profile(str(output_dir).encode())
                if n < 0:
                    raise RuntimeError(f"axon_stop_nrt_profile rc={n}")
                print(f"profile: {n} file(s) written to {output_dir}")

        return _hook

    def set_axon_ntff_profile_hook(hook):
        state["hook"] = hook
        state["inited"] = True

    def get_axon_ntff_profile_hook():
        if not state["inited"]:
            state["hook"] = _default_hook()
            state["inited"] = True
        return state["hook"]

    mod.set_axon_ntff_profile_hook = set_axon_ntff_profile_hook
    mod.get_axon_ntff_profile_hook = get_axon_ntff_profile_hook
    sys.modules["antenv.axon_hooks"] = mod


def kernel(x, w_qkv, b_qkv, w_proj, b_proj, _trace=False, _trace_kwargs=None):
    if _trace:
        _install_ntff_hook_module()
    from concourse.bass_utils import run_bass_kernel_spmd

    x = np.asarray(x, dtype=np.float32)
    b, c, h, w = x.shape
    assert (b, c, h, w) == (B, C, 32, 32)

    weights, bqkc, b_proj_f = _pack_weights(w_qkv, b_qkv, w_proj, b_proj)
    weights = {k: _bf16(v) for k, v in weights.items()}
    weights["bqkc"] = np.ascontiguousarray(bqkc, dtype=np.float32)
    nc = _get_nc()

    in_maps = []
    for core in range(N_CORES):
        m = dict(weights)
        xm = np.ascontiguousarray(x[core].reshape(C, L))
        m["x"] = _bf16(xm)
        m["xf"] = xm + b_proj_f[:, None].astype(np.float32)
        in_maps.append(m)

    res = run_bass_kernel_spmd(
        nc,
        in_maps,
        list(range(N_CORES)),
        trace=_trace,
        **(_trace_kwargs or {}),
    )
    out = np.stack([res.results[core]["out"] for core in range(N_CORES)])
    if _trace:
        _CACHE["last_result"] = res
    return out.reshape(B, C, 32, 32)


# revision 12
# speedup vs baseline: 1.2721x; 1.2721x over previous
"""MHSA block (b=8, c=256, h=w=32, nh=8) on 8 Trainium2 cores.

Sharding: pure data parallel -- one batch element per NeuronCore, no
collectives.  Per-core algorithm (X = x[b] as (C=256, L=1024)):

  QK   = Wqk @ X (+bqk via ACT bias on psum->sbuf copy)   (512, L)
  V^T  = X^T @ Wv^T + bv  (head-padded, ones col h*33+32) (L, 264)
  S^T_h = K_h^T Q_h  (per head, j on partitions)          -> exp(scale*S^T) = P^T
  [O_h; l_h] = V_aug,h^T.T @ P^T_h   (PSUM accumulate)    row 32/96 = softmax denom
  O_norm = O * replicate(1/l)        (PE K=2 matmul replication)
  out  = (x + bproj) + Wproj @ O_norm   (bproj folded into residual host-side)

All matmul operands are bf16 (hw accumulates fp32 in PSUM); exp on ScalarE is
the throughput floor (~8.4M elems/core, ~70us).  Two heads share each S^T PE
pass via 32-row groups.  Small filler matmuls pad PE idle gaps so the HAM
clock-gate keeps the PE at 2.4 GHz through the exp-gated phase.
Normalization + projection are pipelined per 512-column half-stream.
"""

import sys
import os

sys.path.insert(0, "/opt/trn_rl_repo")

from contextlib import ExitStack

import numpy as np

NH, DH, C, L = 8, 32, 256, 1024
B = 8
SCALE = DH ** -0.5
N_CORES = 8


_CACHE = {}


def _build_nc():
    import concourse.tile as tile
    from concourse import bacc, mybir

    f32 = mybir.dt.float32
    bf16 = mybir.dt.bfloat16
    Exp = mybir.ActivationFunctionType.Exp
    Identity = mybir.ActivationFunctionType.Identity

    nc = bacc.Bacc("TRN2", target_bir_lowering=False, debug=False)

    x_d = nc.dram_tensor("x", [C, L], bf16, kind="ExternalInput").ap()
    xf_d = nc.dram_tensor("xf", [C, L], f32, kind="ExternalInput").ap()
    wqkT_d = nc.dram_tensor("wqkT", [C, 512], bf16, kind="ExternalInput").ap()
    bqkc_d = nc.dram_tensor("bqkc", [128, 4], f32, kind="ExternalInput").ap()
    wvT_d = nc.dram_tensor("wvT", [C, 264], bf16, kind="ExternalInput").ap()
    wpT_d = nc.dram_tensor("wpT", [512, 256], bf16, kind="ExternalInput").ap()
    e_d = nc.dram_tensor("ee", [2, 128], bf16, kind="ExternalInput").ap()
    ones_d = nc.dram_tensor("ones_in", [1, 512], bf16, kind="ExternalInput").ap()
    out_d = nc.dram_tensor("out", [C, L], f32, kind="ExternalOutput").ap()

    with tile.TileContext(nc) as tc, ExitStack() as ctx:
        persist = ctx.enter_context(tc.tile_pool(name="persist", bufs=1))
        ptpool = ctx.enter_context(tc.tile_pool(name="pt", bufs=3))
        onpool = ctx.enter_context(tc.tile_pool(name="on", bufs=2))
        smallp = ctx.enter_context(tc.tile_pool(name="small", bufs=2))
        stps = ctx.enter_context(tc.tile_pool(name="stps", bufs=2, space="PSUM"))
        pvps = ctx.enter_context(tc.tile_pool(name="pvps", bufs=1, space="PSUM"))

        # ---- constants / inputs to SBUF ----
        ones = persist.tile([1, 512], bf16, tag="ones", name="ones")
        nc.sync.dma_start(ones[:], ones_d[:])

        # warm the ACT exp table while the QKV phase runs
        warm = persist.tile([1, 8], f32, tag="warm", name="warm")
        nc.gpsimd.memset(warm[:], 0.0)
        nc.scalar.activation(warm[:], warm[:], Exp)

        x_sb = []
        for t in range(2):
            xt = persist.tile([128, L], bf16, tag=f"x{t}", name=f"x{t}")
            nc.sync.dma_start(xt[:], x_d[t * 128:(t + 1) * 128, :])
            x_sb.append(xt)

        wqk_sb = []
        for t in range(2):
            w = persist.tile([128, 512], bf16, tag=f"wqk{t}", name=f"wqk{t}")
            nc.sync.dma_start(w[:], wqkT_d[t * 128:(t + 1) * 128, :])
            wqk_sb.append(w)
        bqkc_sb = persist.tile([128, 4], f32, tag="bqkc", name="bqkc")
        nc.sync.dma_start(bqkc_sb[:], bqkc_d[:])

        wv_sb = []
        for t in range(2):
            w = persist.tile([128, 264], bf16, tag=f"wv{t}", name=f"wv{t}")
            nc.sync.dma_start(w[:], wvT_d[t * 128:(t + 1) * 128, :])
            wv_sb.append(w)

        wp_sb = []
        for p in range(4):
            w = persist.tile([128, 256], bf16, tag=f"wp{p}", name=f"wp{p}")
            nc.sync.dma_start(w[:], wpT_d[p * 128:(p + 1) * 128, :])
            wp_sb.append(w)

        e_sb = persist.tile([2, 128], bf16, tag="ee", name="ee")
        nc.sync.dma_start(e_sb[:], e_d[:])

        o_sb = []
        for p in range(4):
            o = persist.tile([128, L], f32, tag=f"o{p}", name=f"o{p}")
            nc.gpsimd.memset(o[:], 0.0)
            o_sb.append(o)

        acc = [persist.tile([128, L], f32, tag=f"acc{t}", name=f"acc{t}") for t in range(2)]

        xf_sb = []
        for t in range(2):
            xft = persist.tile([128, L], f32, tag=f"xf{t}", name=f"xf{t}")
            nc.sync.dma_start(xft[:], xf_d[t * 128:(t + 1) * 128, :])
            xf_sb.append(xft)

        # ---- QK gemm:  QK(512, L) = WqkT.T @ X; bqk added on the ACT copy ----
        qk_sb = [None] * 4

        def qk_chunk(mt):
            ps = stps.tile([128, L], f32, tag="st", name="qkps")
            for nh_ in range(2):
                o = ps[:, nh_ * 512:(nh_ + 1) * 512]
                for kt in range(2):
                    nc.tensor.matmul(
                        o,
                        lhsT=wqk_sb[kt][:, mt * 128:(mt + 1) * 128],
                        rhs=x_sb[kt][:, nh_ * 512:(nh_ + 1) * 512],
                        start=(kt == 0),
                        stop=(kt == 1),
                    )
            qk = persist.tile([128, L], bf16, tag=f"qk{mt}", name=f"qk{mt}")
            nc.scalar.activation(qk[:], ps[:], Identity, bias=bqkc_sb[:, mt:mt + 1])
            qk_sb[mt] = qk

        # ---- V^T gemm: VT(L, 264) = X.T @ WvT + bv  (head-padded cols) ----
        vt_sb = [None] * 8

        def vt_chunk(jt):
            ps = pvps.tile([128, 264], f32, tag="pv0", name="vtps")
            for kt in range(2):
                nc.tensor.matmul(
                    ps[:],
                    lhsT=x_sb[kt][:, jt * 128:(jt + 1) * 128],
                    rhs=wv_sb[kt][:],
                    start=(kt == 0),
                    stop=(kt == 1),
                )
            vt = persist.tile([128, 264], bf16, tag=f"vt{jt}", name=f"vt{jt}")
            nc.vector.tensor_copy(vt[:], ps[:])
            # softmax-denominator ones column per head
            ones_cols = vt[:].rearrange("p (h c) -> p h c", h=8)[:, :, 32:33]
            nc.gpsimd.memset(ones_cols, 1.0)
            vt_sb[jt] = vt

        qk_chunk(0)
        qk_chunk(2)

        def attention_quad(tg, ih):
            qt = qk_sb[tg]
            kt_ = qk_sb[2 + tg]
            heads = [4 * tg + m for m in range(4)]
            cols = slice(ih * 512, (ih + 1) * 512)
            pv = [
                pvps.tile([33, 512], f32, tag=f"pv{m}", name=f"pv{m}")
                for m in range(4)
            ]
            for jc in range(8):
                sts = [
                    stps.tile([128, L], f32, tag="st", name="stA"),
                    stps.tile([128, L], f32, tag="st", name="stB"),
                ]
                for m in range(4):
                    o = 32 * m
                    nc.tensor.matmul(
                        sts[m // 2][:, (m % 2) * 512:(m % 2) * 512 + 512],
                        lhsT=kt_[o:o + 32, jc * 128:(jc + 1) * 128],
                        rhs=qt[o:o + 32, cols],
                        start=True,
                        stop=True,
                        tile_position=(o, 0),
                    )
                pts = []
                for half in range(2):
                    pt = ptpool.tile([128, L], bf16, tag="pt", name="pt")
                    nc.scalar.activation(pt[:], sts[half][:], Exp, scale=SCALE)
                    pts.append(pt)
                for m in range(4):
                    h = heads[m]
                    nc.tensor.matmul(
                        pv[m][:],
                        lhsT=vt_sb[jc][:, h * 33:h * 33 + 33],
                        rhs=pts[m // 2][:, (m % 2) * 512:(m % 2) * 512 + 512],
                        start=(jc == 0),
                        stop=(jc == 7),
                    )
            for m in range(4):
                pr = 2 * tg + m // 2
                ro = 64 * (m % 2)
                nc.vector.tensor_copy(o_sb[pr][ro:ro + 33, cols], pv[m][:])

            # ---- per-half-stream normalization + projection (2 pairs) ----
            for pi, pr in enumerate((2 * tg, 2 * tg + 1)):
                l_sb = smallp.tile([2, 512], f32, tag=f"l{pi}", name="l")
                nc.sync.dma_start(l_sb[0:1, :], o_sb[pr][32:33, cols])
                nc.sync.dma_start(l_sb[1:2, :], o_sb[pr][96:97, cols])
                rl32 = smallp.tile([2, 512], f32, tag=f"rl32{pi}", name="rl32")
                scr = smallp.tile([2, 512], f32, tag=f"rlscratch{pi}", name="rlscratch")
                nc.vector.reciprocal_approx_accurate(rl32[:], l_sb[:], scr[:])
                rl = smallp.tile([2, 512], bf16, tag=f"rl{pi}", name="rl")
                nc.vector.tensor_copy(rl[:], rl32[:])

                rp = pvps.tile([128, 512], f32, tag=f"pv{2 * pi}", name="rp")
                nc.tensor.matmul(rp[:], lhsT=e_sb[:], rhs=rl[:], start=True, stop=True)
                on = onpool.tile([128, 512], bf16, tag="on", name="on")
                nc.vector.tensor_mul(on[:], o_sb[pr][:, cols], rp[:])
                for mt2 in range(2):
                    pj = pvps.tile([128, 512], f32, tag=f"pv{2 * pi + 1}", name="pj")
                    nc.tensor.matmul(
                        pj[:],
                        lhsT=wp_sb[pr][:, mt2 * 128:(mt2 + 1) * 128],
                        rhs=on[:],
                        start=True,
                        stop=True,
                    )
                    if pr == 0:
                        nc.vector.tensor_add(acc[mt2][:, cols], xf_sb[mt2][:, cols], pj[:])
                    else:
                        nc.vector.tensor_add(acc[mt2][:, cols], acc[mt2][:, cols], pj[:])
                    if pr == 3:
                        nc.sync.dma_start(out_d[mt2 * 128:(mt2 + 1) * 128, cols], acc[mt2][:, cols])

        for jt in range(8):
            vt_chunk(jt)
        attention_quad(0, 0)
        qk_chunk(1)
        qk_chunk(3)
        attention_quad(0, 1)
        attention_quad(1, 0)
        attention_quad(1, 1)

    nc.compile()
    return nc


def _get_nc():
    if "nc" not in _CACHE:
        _CACHE["nc"] = _build_nc()
    return _CACHE["nc"]


def _pack_weights(w_qkv, b_qkv, w_proj, b_proj):
    w_qkv = np.asarray(w_qkv, dtype=np.float32)
    b_qkv = np.asarray(b_qkv, dtype=np.float32)
    w_proj = np.asarray(w_proj, dtype=np.float32)
    b_proj = np.asarray(b_proj, dtype=np.float32)

    wqkT = np.ascontiguousarray(w_qkv[:512].T)                  # (256, 512)
    bqkc = np.ascontiguousarray(b_qkv[:512].reshape(4, 128).T)  # (128, 4)

    wvT = np.zeros((C, 264), dtype=np.float32)
    for h in range(NH):
        wvT[:, h * 33:h * 33 + 32] = w_qkv[512 + h * 32:512 + (h + 1) * 32].T

    # o_sb row layout per pair tile p: head 2p at rows 0:32 (l at 32),
    # head 2p+1 at rows 64:96 (l at 96); all other rows zero.
    wpT = np.zeros((512, 256), dtype=np.float32)
    for p in range(4):
        wpT[p * 128 + 0:p * 128 + 32, :] = w_proj[:, (2 * p) * 32:(2 * p + 1) * 32].T
        wpT[p * 128 + 64:p * 128 + 96, :] = w_proj[:, (2 * p + 1) * 32:(2 * p + 2) * 32].T

    ee = np.zeros((2, 128), dtype=np.float32)
    ee[0, 0:32] = 1.0
    ee[1, 64:96] = 1.0
    ones_in = np.ones((1, 512), dtype=np.float32)
    # residual carries x + bproj + Wproj @ bv (the V-bias contribution:
    # O_norm = O/l + bv, and Wproj @ bv is column-constant)
    resid_bias = b_proj + w_proj @ b_qkv[512:768]
    return dict(wqkT=wqkT, wvT=wvT, wpT=wpT, ee=ee,
                ones_in=ones_in), bqkc, resid_bias


def _bf16(a):
    import ml_dtypes

    return np.asarray(a).astype(ml_dtypes.bfloat16)


def _install_ntff_hook_module():
    """bass_utils wants antenv.axon_hooks for trace=True under axon; this
    image's antenv lacks it.  Inject an equivalent module into sys.modules."""
    if "antenv.axon_hooks" in sys.modules:
        return
    try:
        import antenv.axon_hooks  # noqa: F401

        return
    except ImportError:
        pass
    import contextlib
    import ctypes
    import types

    mod = types.ModuleType("antenv.axon_hooks")
    state = {"hook": None, "inited": False}

    def _default_hook():
        so_path = "/opt/axon/libaxon_pjrt.so"
        if not os.path.exists(so_path):
            return None
        lib = ctypes.CDLL(so_path)
        if not hasattr(lib, "axon_start_nrt_profile"):
            return None
        lib.axon_start_nrt_profile.argtypes = [
            ctypes.POINTER(ctypes.c_int64),
            ctypes.c_size_t,
        ]
        lib.axon_start_nrt_profile.restype = ctypes.c_int64
        lib.axon_stop_nrt_profile.argtypes = [ctypes.c_char_p]
        lib.axon_stop_nrt_profile.restype = ctypes.c_int64

        @contextlib.contextmanager
        def _hook(output_dir, device_ids):
            import jax

            jax.devices()
            if device_ids:
                ids = (ctypes.c_int64 * len(device_ids))(*device_ids)
                rc = lib.axon_start_nrt_profile(ids, len(device_ids))
            else:
                rc = lib.axon_start_nrt_profile(None, 0)
            if rc != 0:
                raise RuntimeError(f"axon_start_nrt_profile rc={rc}")
            try:
                yield
            finally:
                n = lib.axon_stop_nrt_profile(str(output_dir).encode())
                if n < 0:
                    raise RuntimeError(f"axon_stop_nrt_profile rc={n}")
                print(f"profile: {n} file(s) written to {output_dir}")

        return _hook

    def set_axon_ntff_profile_hook(hook):
        state["hook"] = hook
        state["inited"] = True

    def get_axon_ntff_profile_hook():
        if not state["inited"]:
            state["hook"] = _default_hook()
            state["inited"] = True
        return state["hook"]

    mod.set_axon_ntff_profile_hook = set_axon_ntff_profile_hook
    mod.get_axon_ntff_profile_hook = get_axon_ntff_profile_hook
    sys.modules["antenv.axon_hooks"] = mod


def kernel(x, w_qkv, b_qkv, w_proj, b_proj, _trace=False, _trace_kwargs=None):
    if _trace:
        _install_ntff_hook_module()
    from concourse.bass_utils import run_bass_kernel_spmd

    x = np.asarray(x, dtype=np.float32)
    b, c, h, w = x.shape
    assert (b, c, h, w) == (B, C, 32, 32)

    weights, bqkc, resid_bias = _pack_weights(w_qkv, b_qkv, w_proj, b_proj)
    weights = {k: _bf16(v) for k, v in weights.items()}
    weights["bqkc"] = np.ascontiguousarray(bqkc, dtype=np.float32)
    nc = _get_nc()

    in_maps = []
    for core in range(N_CORES):
        m = dict(weights)
        xm = np.ascontiguousarray(x[core].reshape(C, L))
        m["x"] = _bf16(xm)
        m["xf"] = xm + resid_bias[:, None].astype(np.float32)
        in_maps.append(m)

    res = run_bass_kernel_spmd(
        nc,
        in_maps,
        list(range(N_CORES)),
        trace=_trace,
        **(_trace_kwargs or {}),
    )
    out = np.stack([res.results[core]["out"] for core in range(N_CORES)])
    if _trace:
        _CACHE["last_result"] = res
    return out.reshape(B, C, 32, 32)


# revision 13
# speedup vs baseline: 1.3584x; 1.0679x over previous
"""MHSA block (b=8, c=256, h=w=32, nh=8) on 8 Trainium2 cores.

Sharding: pure data parallel -- one batch element per NeuronCore, no
collectives.  Per-core algorithm (X = x[b] as (C=256, L=1024)):

  QK   = Wqk @ X (+bqk via ACT bias on psum->sbuf copy)   (512, L)
  V^T  = X^T @ Wv^T + bv  (head-padded, ones col h*33+32) (L, 264)
  S^T_h = K_h^T Q_h  (per head, j on partitions)          -> exp(scale*S^T) = P^T
  [O_h; l_h] = V_aug,h^T.T @ P^T_h   (PSUM accumulate)    row 32/96 = softmax denom
  O_norm = O * replicate(1/l)        (PE K=2 matmul replication)
  out  = (x + bproj) + Wproj @ O_norm   (bproj folded into residual host-side)

All matmul operands are bf16 (hw accumulates fp32 in PSUM); exp on ScalarE is
the throughput floor (~8.4M elems/core, ~70us).  Two heads share each S^T PE
pass via 32-row groups.  Small filler matmuls pad PE idle gaps so the HAM
clock-gate keeps the PE at 2.4 GHz through the exp-gated phase.
Normalization + projection are pipelined per 512-column half-stream.
"""

import sys
import os

sys.path.insert(0, "/opt/trn_rl_repo")

from contextlib import ExitStack

import numpy as np

NH, DH, C, L = 8, 32, 256, 1024
B = 8
SCALE = DH ** -0.5
N_CORES = 8


_CACHE = {}


def _build_nc():
    import concourse.tile as tile
    from concourse import bacc, mybir

    f32 = mybir.dt.float32
    bf16 = mybir.dt.bfloat16
    Exp = mybir.ActivationFunctionType.Exp
    Identity = mybir.ActivationFunctionType.Identity

    nc = bacc.Bacc("TRN2", target_bir_lowering=False, debug=False)

    x_d = nc.dram_tensor("x", [C, L], bf16, kind="ExternalInput").ap()
    xf_d = nc.dram_tensor("xf", [C, L], f32, kind="ExternalInput").ap()
    wqkT_d = nc.dram_tensor("wqkT", [C, 512], bf16, kind="ExternalInput").ap()
    bqkc_d = nc.dram_tensor("bqkc", [128, 4], f32, kind="ExternalInput").ap()
    wvT_d = nc.dram_tensor("wvT", [C, 264], bf16, kind="ExternalInput").ap()
    wpT_d = nc.dram_tensor("wpT", [512, 256], bf16, kind="ExternalInput").ap()
    e_d = nc.dram_tensor("ee", [2, 128], bf16, kind="ExternalInput").ap()
    ones_d = nc.dram_tensor("ones_in", [1, 512], bf16, kind="ExternalInput").ap()
    out_d = nc.dram_tensor("out", [C, L], f32, kind="ExternalOutput").ap()

    with tile.TileContext(nc) as tc, ExitStack() as ctx:
        persist = ctx.enter_context(tc.tile_pool(name="persist", bufs=1))
        ptpool = ctx.enter_context(tc.tile_pool(name="pt", bufs=3))
        onpool = ctx.enter_context(tc.tile_pool(name="on", bufs=2))
        smallp = ctx.enter_context(tc.tile_pool(name="small", bufs=2))
        stps = ctx.enter_context(tc.tile_pool(name="stps", bufs=2, space="PSUM"))
        pvps = ctx.enter_context(tc.tile_pool(name="pvps", bufs=1, space="PSUM"))

        # ---- constants / inputs to SBUF ----
        ones = persist.tile([1, 512], bf16, tag="ones", name="ones")
        nc.sync.dma_start(ones[:], ones_d[:])

        # warm the ACT exp table while the QKV phase runs
        warm = persist.tile([1, 8], f32, tag="warm", name="warm")
        nc.gpsimd.memset(warm[:], 0.0)
        nc.scalar.activation(warm[:], warm[:], Exp)

        x_sb = []
        for t in range(2):
            xt = persist.tile([128, L], bf16, tag=f"x{t}", name=f"x{t}")
            nc.sync.dma_start(xt[:], x_d[t * 128:(t + 1) * 128, :])
            x_sb.append(xt)

        wqk_sb = []
        for t in range(2):
            w = persist.tile([128, 512], bf16, tag=f"wqk{t}", name=f"wqk{t}")
            nc.sync.dma_start(w[:], wqkT_d[t * 128:(t + 1) * 128, :])
            wqk_sb.append(w)
        bqkc_sb = persist.tile([128, 4], f32, tag="bqkc", name="bqkc")
        nc.sync.dma_start(bqkc_sb[:], bqkc_d[:])

        # ---- QK gemm:  QK(512, L) = WqkT.T @ X; bqk added on the ACT copy ----
        qk_sb = [None] * 4

        def qk_chunk(mt):
            ps = stps.tile([128, L], f32, tag="st", name="qkps")
            for nh_ in range(2):
                o = ps[:, nh_ * 512:(nh_ + 1) * 512]
                for kt in range(2):
                    nc.tensor.matmul(
                        o,
                        lhsT=wqk_sb[kt][:, mt * 128:(mt + 1) * 128],
                        rhs=x_sb[kt][:, nh_ * 512:(nh_ + 1) * 512],
                        start=(kt == 0),
                        stop=(kt == 1),
                    )
            qk = persist.tile([128, L], bf16, tag=f"qk{mt}", name=f"qk{mt}")
            nc.scalar.activation(qk[:], ps[:], Identity, bias=bqkc_sb[:, mt:mt + 1])
            qk_sb[mt] = qk

        # ---- V^T gemm: VT(L, 264) = X.T @ WvT + bv  (head-padded cols) ----
        vt_sb = [None] * 8

        def vt_chunk(jt):
            ps = pvps.tile([128, 264], f32, tag="pv0", name="vtps")
            for kt in range(2):
                nc.tensor.matmul(
                    ps[:],
                    lhsT=x_sb[kt][:, jt * 128:(jt + 1) * 128],
                    rhs=wv_sb[kt][:],
                    start=(kt == 0),
                    stop=(kt == 1),
                )
            vt = persist.tile([128, 264], bf16, tag=f"vt{jt}", name=f"vt{jt}")
            nc.vector.tensor_copy(vt[:], ps[:])
            # softmax-denominator ones column per head
            ones_cols = vt[:].rearrange("p (h c) -> p h c", h=8)[:, :, 32:33]
            nc.gpsimd.memset(ones_cols, 1.0)
            vt_sb[jt] = vt

        qk_chunk(0)
        qk_chunk(2)

        wv_sb = []
        for t in range(2):
            w = persist.tile([128, 264], bf16, tag=f"wv{t}", name=f"wv{t}")
            nc.sync.dma_start(w[:], wvT_d[t * 128:(t + 1) * 128, :])
            wv_sb.append(w)

        o_sb = []
        for p in range(4):
            o = persist.tile([128, L], f32, tag=f"o{p}", name=f"o{p}")
            nc.gpsimd.memset(o[:], 0.0)
            o_sb.append(o)

        e_sb = persist.tile([2, 128], bf16, tag="ee", name="ee")
        nc.sync.dma_start(e_sb[:], e_d[:])

        wp_sb = []
        for p in range(4):
            w = persist.tile([128, 256], bf16, tag=f"wp{p}", name=f"wp{p}")
            nc.sync.dma_start(w[:], wpT_d[p * 128:(p + 1) * 128, :])
            wp_sb.append(w)

        acc = [persist.tile([128, L], f32, tag=f"acc{t}", name=f"acc{t}") for t in range(2)]

        xf_sb = []
        for t in range(2):
            xft = persist.tile([128, L], f32, tag=f"xf{t}", name=f"xf{t}")
            nc.sync.dma_start(xft[:], xf_d[t * 128:(t + 1) * 128, :])
            xf_sb.append(xft)

        def attention_quad(tg, ih):
            qt = qk_sb[tg]
            kt_ = qk_sb[2 + tg]
            heads = [4 * tg + m for m in range(4)]
            cols = slice(ih * 512, (ih + 1) * 512)
            pv = [
                pvps.tile([33, 512], f32, tag=f"pv{m}", name=f"pv{m}")
                for m in range(4)
            ]
            for jc in range(8):
                if jc == 2 and deferred:
                    deferred.pop(0)()
                sts = [
                    stps.tile([128, L], f32, tag="st", name="stA"),
                    stps.tile([128, L], f32, tag="st", name="stB"),
                ]
                for m in range(4):
                    o = 32 * m
                    nc.tensor.matmul(
                        sts[m // 2][:, (m % 2) * 512:(m % 2) * 512 + 512],
                        lhsT=kt_[o:o + 32, jc * 128:(jc + 1) * 128],
                        rhs=qt[o:o + 32, cols],
                        start=True,
                        stop=True,
                        tile_position=(o, 0),
                    )
                pts = []
                for half in range(2):
                    pt = ptpool.tile([128, L], bf16, tag="pt", name="pt")
                    nc.scalar.activation(pt[:], sts[half][:], Exp, scale=SCALE)
                    pts.append(pt)
                for m in range(4):
                    h = heads[m]
                    nc.tensor.matmul(
                        pv[m][:],
                        lhsT=vt_sb[jc][:, h * 33:h * 33 + 33],
                        rhs=pts[m // 2][:, (m % 2) * 512:(m % 2) * 512 + 512],
                        start=(jc == 0),
                        stop=(jc == 7),
                    )
            for m in range(4):
                pr = 2 * tg + m // 2
                ro = 64 * (m % 2)
                nc.vector.tensor_copy(o_sb[pr][ro:ro + 33, cols], pv[m][:])

            def norm_chain(tg=tg, ih=ih, cols=cols):
                for pi, pr in enumerate((2 * tg, 2 * tg + 1)):
                    l_sb = smallp.tile([2, 512], f32, tag=f"l{pi}", name="l")
                    nc.sync.dma_start(l_sb[0:1, :], o_sb[pr][32:33, cols])
                    nc.sync.dma_start(l_sb[1:2, :], o_sb[pr][96:97, cols])
                    rl32 = smallp.tile([2, 512], f32, tag=f"rl32{pi}", name="rl32")
                    scr = smallp.tile([2, 512], f32, tag=f"rls{pi}", name="rlscratch")
                    nc.vector.reciprocal_approx_accurate(rl32[:], l_sb[:], scr[:])
                    rl = smallp.tile([2, 512], bf16, tag=f"rl{pi}", name="rl")
                    nc.vector.tensor_copy(rl[:], rl32[:])

                    rp = stps.tile([128, 512], f32, tag="st", name="rp")
                    nc.tensor.matmul(rp[:], lhsT=e_sb[:], rhs=rl[:], start=True, stop=True)
                    on = onpool.tile([128, 512], bf16, tag="on", name="on")
                    nc.vector.tensor_mul(on[:], o_sb[pr][:, cols], rp[:])
                    for mt2 in range(2):
                        pj = stps.tile([128, 512], f32, tag="st", name="pj")
                        nc.tensor.matmul(
                            pj[:],
                            lhsT=wp_sb[pr][:, mt2 * 128:(mt2 + 1) * 128],
                            rhs=on[:],
                            start=True,
                            stop=True,
                        )
                        if pr == 0:
                            nc.vector.tensor_add(acc[mt2][:, cols], xf_sb[mt2][:, cols], pj[:])
                        else:
                            nc.vector.tensor_add(acc[mt2][:, cols], acc[mt2][:, cols], pj[:])
                        if pr == 3:
                            nc.sync.dma_start(out_d[mt2 * 128:(mt2 + 1) * 128, cols], acc[mt2][:, cols])

            deferred.append(norm_chain)

        deferred = []
        for jt in range(8):
            vt_chunk(jt)
        attention_quad(0, 0)
        qk_chunk(1)
        qk_chunk(3)
        attention_quad(0, 1)
        attention_quad(1, 0)
        attention_quad(1, 1)
        while deferred:
            deferred.pop(0)()

    nc.compile()
    return nc


def _get_nc():
    if "nc" not in _CACHE:
        _CACHE["nc"] = _build_nc()
    return _CACHE["nc"]


def _pack_weights(w_qkv, b_qkv, w_proj, b_proj):
    w_qkv = np.asarray(w_qkv, dtype=np.float32)
    b_qkv = np.asarray(b_qkv, dtype=np.float32)
    w_proj = np.asarray(w_proj, dtype=np.float32)
    b_proj = np.asarray(b_proj, dtype=np.float32)

    wqkT = np.ascontiguousarray(w_qkv[:512].T)                  # (256, 512)
    bqkc = np.ascontiguousarray(b_qkv[:512].reshape(4, 128).T)  # (128, 4)

    wvT = np.zeros((C, 264), dtype=np.float32)
    for h in range(NH):
        wvT[:, h * 33:h * 33 + 32] = w_qkv[512 + h * 32:512 + (h + 1) * 32].T

    # o_sb row layout per pair tile p: head 2p at rows 0:32 (l at 32),
    # head 2p+1 at rows 64:96 (l at 96); all other rows zero.
    wpT = np.zeros((512, 256), dtype=np.float32)
    for p in range(4):
        wpT[p * 128 + 0:p * 128 + 32, :] = w_proj[:, (2 * p) * 32:(2 * p + 1) * 32].T
        wpT[p * 128 + 64:p * 128 + 96, :] = w_proj[:, (2 * p + 1) * 32:(2 * p + 2) * 32].T

    ee = np.zeros((2, 128), dtype=np.float32)
    ee[0, 0:32] = 1.0
    ee[1, 64:96] = 1.0
    ones_in = np.ones((1, 512), dtype=np.float32)
    # residual carries x + bproj + Wproj @ bv (the V-bias contribution:
    # O_norm = O/l + bv, and Wproj @ bv is column-constant)
    resid_bias = b_proj + w_proj @ b_qkv[512:768]
    return dict(wqkT=wqkT, wvT=wvT, wpT=wpT, ee=ee,
                ones_in=ones_in), bqkc, resid_bias


def _bf16(a):
    import ml_dtypes

    return np.asarray(a).astype(ml_dtypes.bfloat16)


def _install_ntff_hook_module():
    """bass_utils wants antenv.axon_hooks for trace=True under axon; this
    image's antenv lacks it.  Inject an equivalent module into sys.modules."""
    if "antenv.axon_hooks" in sys.modules:
        return
    try:
        import antenv.axon_hooks  # noqa: F401

        return
    except ImportError:
        pass
    import contextlib
    import ctypes
    import types

    mod = types.ModuleType("antenv.axon_hooks")
    state = {"hook": None, "inited": False}

    def _default_hook():
        so_path = "/opt/axon/libaxon_pjrt.so"
        if not os.path.exists(so_path):
            return None
        lib = ctypes.CDLL(so_path)
        if not hasattr(lib, "axon_start_nrt_profile"):
            return None
        lib.axon_start_nrt_profile.argtypes = [
            ctypes.POINTER(ctypes.c_int64),
            ctypes.c_size_t,
        ]
        lib.axon_start_nrt_profile.restype = ctypes.c_int64
        lib.axon_stop_nrt_profile.argtypes = [ctypes.c_char_p]
        lib.axon_stop_nrt_profile.restype = ctypes.c_int64

        @contextlib.contextmanager
        def _hook(output_dir, device_ids):
            import jax

            jax.devices()
            if device_ids:
                ids = (ctypes.c_int64 * len(device_ids))(*device_ids)
                rc = lib.axon_start_nrt_profile(ids, len(device_ids))
            else:
                rc = lib.axon_start_nrt_profile(None, 0)
            if rc != 0:
                raise RuntimeError(f"axon_start_nrt_profile rc={rc}")
            try:
                yield
            finally:
                n = lib.axon_stop_nrt_profile(str(output_dir).encode())
                if n < 0:
                    raise RuntimeError(f"axon_stop_nrt_profile rc={n}")
                print(f"profile: {n} file(s) written to {output_dir}")

        return _hook

    def set_axon_ntff_profile_hook(hook):
        state["hook"] = hook
        state["inited"] = True

    def get_axon_ntff_profile_hook():
        if not state["inited"]:
            state["hook"] = _default_hook()
            state["inited"] = True
        return state["hook"]

    mod.set_axon_ntff_profile_hook = set_axon_ntff_profile_hook
    mod.get_axon_ntff_profile_hook = get_axon_ntff_profile_hook
    sys.modules["antenv.axon_hooks"] = mod


def kernel(x, w_qkv, b_qkv, w_proj, b_proj, _trace=False, _trace_kwargs=None):
    if _trace:
        _install_ntff_hook_module()
    from concourse.bass_utils import run_bass_kernel_spmd

    x = np.asarray(x, dtype=np.float32)
    b, c, h, w = x.shape
    assert (b, c, h, w) == (B, C, 32, 32)

    weights, bqkc, resid_bias = _pack_weights(w_qkv, b_qkv, w_proj, b_proj)
    weights = {k: _bf16(v) for k, v in weights.items()}
    weights["bqkc"] = np.ascontiguousarray(bqkc, dtype=np.float32)
    nc = _get_nc()

    in_maps = []
    for core in range(N_CORES):
        m = dict(weights)
        xm = np.ascontiguousarray(x[core].reshape(C, L))
        m["x"] = _bf16(xm)
        m["xf"] = xm + resid_bias[:, None].astype(np.float32)
        in_maps.append(m)

    res = run_bass_kernel_spmd(
        nc,
        in_maps,
        list(range(N_CORES)),
        trace=_trace,
        **(_trace_kwargs or {}),
    )
    out = np.stack([res.results[core]["out"] for core in range(N_CORES)])
    if _trace:
        _CACHE["last_result"] = res
    return out.reshape(B, C, 32, 32)


# revision 14
# speedup vs baseline: 1.3716x; 1.0097x over previous
"""MHSA block (b=8, c=256, h=w=32, nh=8) on 8 Trainium2 cores.

Sharding: pure data parallel -- one batch element per NeuronCore, no
collectives.  Per-core algorithm (X = x[b] as (C=256, L=1024)):

  QK   = Wqk @ X (+bqk via ACT bias on psum->sbuf copy)   (512, L)
  V^T  = X^T @ Wv^T + bv  (head-padded, ones col h*33+32) (L, 264)
  S^T_h = K_h^T Q_h  (per head, j on partitions)          -> exp(scale*S^T) = P^T
  [O_h; l_h] = V_aug,h^T.T @ P^T_h   (PSUM accumulate)    row 32/96 = softmax denom
  O_norm = O * replicate(1/l)        (PE K=2 matmul replication)
  out  = (x + bproj) + Wproj @ O_norm   (bproj folded into residual host-side)

All matmul operands are bf16 (hw accumulates fp32 in PSUM); exp on ScalarE is
the throughput floor (~8.4M elems/core, ~70us).  Two heads share each S^T PE
pass via 32-row groups.  Small filler matmuls pad PE idle gaps so the HAM
clock-gate keeps the PE at 2.4 GHz through the exp-gated phase.
Normalization + projection are pipelined per 512-column half-stream.
"""

import sys
import os

sys.path.insert(0, "/opt/trn_rl_repo")

from contextlib import ExitStack

import numpy as np

NH, DH, C, L = 8, 32, 256, 1024
B = 8
SCALE = DH ** -0.5
N_CORES = 8


_CACHE = {}


def _build_nc():
    import concourse.tile as tile
    from concourse import bacc, mybir

    f32 = mybir.dt.float32
    bf16 = mybir.dt.bfloat16
    Exp = mybir.ActivationFunctionType.Exp
    Identity = mybir.ActivationFunctionType.Identity

    nc = bacc.Bacc("TRN2", target_bir_lowering=False, debug=False)

    x_d = nc.dram_tensor("x", [C, L], bf16, kind="ExternalInput").ap()
    xf_d = nc.dram_tensor("xf", [C, L], f32, kind="ExternalInput").ap()
    wblob_d = nc.dram_tensor("wblob", [128, 2576], bf16, kind="ExternalInput").ap()
    bqkc_d = nc.dram_tensor("bqkc", [128, 4], f32, kind="ExternalInput").ap()
    e_d = nc.dram_tensor("ee", [2, 128], bf16, kind="ExternalInput").ap()
    ones_d = nc.dram_tensor("ones_in", [1, 512], bf16, kind="ExternalInput").ap()
    out_d = nc.dram_tensor("out", [C, L], f32, kind="ExternalOutput").ap()

    with tile.TileContext(nc) as tc, ExitStack() as ctx:
        persist = ctx.enter_context(tc.tile_pool(name="persist", bufs=1))
        ptpool = ctx.enter_context(tc.tile_pool(name="pt", bufs=3))
        onpool = ctx.enter_context(tc.tile_pool(name="on", bufs=2))
        smallp = ctx.enter_context(tc.tile_pool(name="small", bufs=2))
        stps = ctx.enter_context(tc.tile_pool(name="stps", bufs=2, space="PSUM"))
        pvps = ctx.enter_context(tc.tile_pool(name="pvps", bufs=1, space="PSUM"))

        # ---- constants / inputs to SBUF (x + weights blob first) ----
        x_sb = []
        for t in range(2):
            xt = persist.tile([128, L], bf16, tag=f"x{t}", name=f"x{t}")
            nc.sync.dma_start(xt[:], x_d[t * 128:(t + 1) * 128, :])
            x_sb.append(xt)

        wblob = persist.tile([128, 2576], bf16, tag="wblob", name="wblob")
        nc.sync.dma_start(wblob[:], wblob_d[:])
        wqk_sb = [wblob[:, 0:512], wblob[:, 512:1024]]
        wv_sb = [wblob[:, 1024:1288], wblob[:, 1288:1552]]
        wp_sb = [wblob[:, 1552 + 256 * p:1552 + 256 * (p + 1)] for p in range(4)]

        bqkc_sb = persist.tile([128, 4], f32, tag="bqkc", name="bqkc")
        nc.sync.dma_start(bqkc_sb[:], bqkc_d[:])
        ones = persist.tile([1, 512], bf16, tag="ones", name="ones")
        nc.sync.dma_start(ones[:], ones_d[:])

        # warm the ACT exp table while the QKV phase runs
        warm = persist.tile([1, 8], f32, tag="warm", name="warm")
        nc.gpsimd.memset(warm[:], 0.0)
        nc.scalar.activation(warm[:], warm[:], Exp)

        # ---- QK gemm:  QK(512, L) = WqkT.T @ X; bqk added on the ACT copy ----
        qk_sb = [None] * 4

        def qk_chunk(mt):
            ps = stps.tile([128, L], f32, tag="st", name="qkps")
            for nh_ in range(2):
                o = ps[:, nh_ * 512:(nh_ + 1) * 512]
                for kt in range(2):
                    nc.tensor.matmul(
                        o,
                        lhsT=wqk_sb[kt][:, mt * 128:(mt + 1) * 128],
                        rhs=x_sb[kt][:, nh_ * 512:(nh_ + 1) * 512],
                        start=(kt == 0),
                        stop=(kt == 1),
                    )
            qk = persist.tile([128, L], bf16, tag=f"qk{mt}", name=f"qk{mt}")
            nc.scalar.activation(qk[:], ps[:], Identity, bias=bqkc_sb[:, mt:mt + 1])
            qk_sb[mt] = qk

        # ---- V^T gemm: VT(L, 264) = X.T @ WvT + bv  (head-padded cols) ----
        vt_sb = [None] * 8

        def vt_chunk(jt):
            ps = pvps.tile([128, 264], f32, tag="pv0", name="vtps")
            for kt in range(2):
                nc.tensor.matmul(
                    ps[:],
                    lhsT=x_sb[kt][:, jt * 128:(jt + 1) * 128],
                    rhs=wv_sb[kt],
                    start=(kt == 0),
                    stop=(kt == 1),
                )
            vt = persist.tile([128, 264], bf16, tag=f"vt{jt}", name=f"vt{jt}")
            nc.vector.tensor_copy(vt[:], ps[:])
            # softmax-denominator ones column per head
            ones_cols = vt[:].rearrange("p (h c) -> p h c", h=8)[:, :, 32:33]
            nc.gpsimd.memset(ones_cols, 1.0)
            vt_sb[jt] = vt

        qk_chunk(0)
        qk_chunk(2)

        o_sb = []
        for p in range(4):
            o = persist.tile([128, L], f32, tag=f"o{p}", name=f"o{p}")
            nc.gpsimd.memset(o[:], 0.0)
            o_sb.append(o)

        e_sb = persist.tile([2, 128], bf16, tag="ee", name="ee")
        nc.sync.dma_start(e_sb[:], e_d[:])

        acc = [persist.tile([128, L], f32, tag=f"acc{t}", name=f"acc{t}") for t in range(2)]

        xf_sb = []
        for t in range(2):
            xft = persist.tile([128, L], f32, tag=f"xf{t}", name=f"xf{t}")
            nc.sync.dma_start(xft[:], xf_d[t * 128:(t + 1) * 128, :])
            xf_sb.append(xft)

        def attention_quad(tg, ih):
            qt = qk_sb[tg]
            kt_ = qk_sb[2 + tg]
            heads = [4 * tg + m for m in range(4)]
            cols = slice(ih * 512, (ih + 1) * 512)
            pv = [
                pvps.tile([33, 512], f32, tag=f"pv{m}", name=f"pv{m}")
                for m in range(4)
            ]
            for jc in range(8):
                if jc == 2 and len(deferred) >= 2:
                    deferred.pop(0)()
                sts = [
                    stps.tile([128, L], f32, tag="st", name="stA"),
                    stps.tile([128, L], f32, tag="st", name="stB"),
                ]
                for m in range(4):
                    o = 32 * m
                    nc.tensor.matmul(
                        sts[m // 2][:, (m % 2) * 512:(m % 2) * 512 + 512],
                        lhsT=kt_[o:o + 32, jc * 128:(jc + 1) * 128],
                        rhs=qt[o:o + 32, cols],
                        start=True,
                        stop=True,
                        tile_position=(o, 0),
                    )
                pts = []
                for half in range(2):
                    pt = ptpool.tile([128, L], bf16, tag="pt", name="pt")
                    nc.scalar.activation(pt[:], sts[half][:], Exp, scale=SCALE)
                    pts.append(pt)
                for m in range(4):
                    h = heads[m]
                    nc.tensor.matmul(
                        pv[m][:],
                        lhsT=vt_sb[jc][:, h * 33:h * 33 + 33],
                        rhs=pts[m // 2][:, (m % 2) * 512:(m % 2) * 512 + 512],
                        start=(jc == 0),
                        stop=(jc == 7),
                    )
            for m in range(4):
                pr = 2 * tg + m // 2
                ro = 64 * (m % 2)
                nc.vector.tensor_copy(o_sb[pr][ro:ro + 33, cols], pv[m][:])

            def norm_chain(tg=tg, ih=ih, cols=cols):
                wk = iter([0, 1, 2, 3, 0, 1])
                for pi, pr in enumerate((2 * tg, 2 * tg + 1)):
                    l_sb = smallp.tile([2, 512], f32, tag=f"l{pi}", name="l")
                    nc.sync.dma_start(l_sb[0:1, :], o_sb[pr][32:33, cols])
                    nc.sync.dma_start(l_sb[1:2, :], o_sb[pr][96:97, cols])
                    rl32 = smallp.tile([2, 512], f32, tag=f"rl32{pi}", name="rl32")
                    nc.vector.reciprocal_approx_fast(rl32[:], l_sb[:])
                    rl = smallp.tile([2, 512], bf16, tag=f"rl{pi}", name="rl")
                    nc.vector.tensor_copy(rl[:], rl32[:])

                    rp = pvps.tile([128, 512], f32, tag=f"pv{next(wk)}", name="rp")
                    nc.tensor.matmul(rp[:], lhsT=e_sb[:], rhs=rl[:], start=True, stop=True)
                    on = onpool.tile([128, 512], bf16, tag="on", name="on")
                    nc.vector.tensor_mul(on[:], o_sb[pr][:, cols], rp[:])
                    for mt2 in range(2):
                        pj = pvps.tile([128, 512], f32, tag=f"pv{next(wk)}", name="pj")
                        nc.tensor.matmul(
                            pj[:],
                            lhsT=wp_sb[pr][:, mt2 * 128:(mt2 + 1) * 128],
                            rhs=on[:],
                            start=True,
                            stop=True,
                        )
                        if pr == 0:
                            nc.vector.tensor_add(acc[mt2][:, cols], xf_sb[mt2][:, cols], pj[:])
                        else:
                            nc.vector.tensor_add(acc[mt2][:, cols], acc[mt2][:, cols], pj[:])
                        if pr == 3:
                            nc.sync.dma_start(out_d[mt2 * 128:(mt2 + 1) * 128, cols], acc[mt2][:, cols])

            deferred.append(norm_chain)

        deferred = []
        for jt in range(8):
            vt_chunk(jt)
        attention_quad(0, 0)
        qk_chunk(1)
        qk_chunk(3)
        attention_quad(0, 1)
        attention_quad(1, 0)
        attention_quad(1, 1)
        while deferred:
            deferred.pop(0)()

    nc.compile()
    return nc


def _get_nc():
    if "nc" not in _CACHE:
        _CACHE["nc"] = _build_nc()
    return _CACHE["nc"]


def _pack_weights(w_qkv, b_qkv, w_proj, b_proj):
    w_qkv = np.asarray(w_qkv, dtype=np.float32)
    b_qkv = np.asarray(b_qkv, dtype=np.float32)
    w_proj = np.asarray(w_proj, dtype=np.float32)
    b_proj = np.asarray(b_proj, dtype=np.float32)

    wqkT = np.ascontiguousarray(w_qkv[:512].T)                  # (256, 512)
    bqkc = np.ascontiguousarray(b_qkv[:512].reshape(4, 128).T)  # (128, 4)

    wvT = np.zeros((C, 264), dtype=np.float32)
    for h in range(NH):
        wvT[:, h * 33:h * 33 + 32] = w_qkv[512 + h * 32:512 + (h + 1) * 32].T

    # o_sb row layout per pair tile p: head 2p at rows 0:32 (l at 32),
    # head 2p+1 at rows 64:96 (l at 96); all other rows zero.
    wpT = np.zeros((512, 256), dtype=np.float32)
    for p in range(4):
        wpT[p * 128 + 0:p * 128 + 32, :] = w_proj[:, (2 * p) * 32:(2 * p + 1) * 32].T
        wpT[p * 128 + 64:p * 128 + 96, :] = w_proj[:, (2 * p + 1) * 32:(2 * p + 2) * 32].T

    ee = np.zeros((2, 128), dtype=np.float32)
    ee[0, 0:32] = 1.0
    ee[1, 64:96] = 1.0
    ones_in = np.ones((1, 512), dtype=np.float32)
    wblob = np.zeros((128, 2576), dtype=np.float32)
    wblob[:, 0:512] = wqkT[0:128]
    wblob[:, 512:1024] = wqkT[128:256]
    wblob[:, 1024:1288] = wvT[0:128]
    wblob[:, 1288:1552] = wvT[128:256]
    for p in range(4):
        wblob[:, 1552 + 256 * p:1552 + 256 * (p + 1)] = wpT[p * 128:(p + 1) * 128]
    # residual carries x + bproj + Wproj @ bv (the V-bias contribution:
    # O_norm = O/l + bv, and Wproj @ bv is column-constant)
    resid_bias = b_proj + w_proj @ b_qkv[512:768]
    return dict(wblob=wblob, ee=ee, ones_in=ones_in), bqkc, resid_bias


def _bf16(a):
    import ml_dtypes

    return np.asarray(a).astype(ml_dtypes.bfloat16)


def _install_ntff_hook_module():
    """bass_utils wants antenv.axon_hooks for trace=True under axon; this
    image's antenv lacks it.  Inject an equivalent module into sys.modules."""
    if "antenv.axon_hooks" in sys.modules:
        return
    try:
        import antenv.axon_hooks  # noqa: F401

        return
    except ImportError:
        pass
    import contextlib
    import ctypes
    import types

    mod = types.ModuleType("antenv.axon_hooks")
    state = {"hook": None, "inited": False}

    def _default_hook():
        so_path = "/opt/axon/libaxon_pjrt.so"
        if not os.path.exists(so_path):
            return None
        lib = ctypes.CDLL(so_path)
        if not hasattr(lib, "axon_start_nrt_profile"):
            return None
        lib.axon_start_nrt_profile.argtypes = [
            ctypes.POINTER(ctypes.c_int64),
            ctypes.c_size_t,
        ]
        lib.axon_start_nrt_profile.restype = ctypes.c_int64
        lib.axon_stop_nrt_profile.argtypes = [ctypes.c_char_p]
        lib.axon_stop_nrt_profile.restype = ctypes.c_int64

        @contextlib.contextmanager
        def _hook(output_dir, device_ids):
            import jax

            jax.devices()
            if device_ids:
                ids = (ctypes.c_int64 * len(device_ids))(*device_ids)
                rc = lib.axon_start_nrt_profile(ids, len(device_ids))
            else:
                rc = lib.axon_start_nrt_profile(None, 0)
            if rc != 0:
                raise RuntimeError(f"axon_start_nrt_profile rc={rc}")
            try:
                yield
            finally:
                n = lib.axon_stop_nrt_profile(str(output_dir).encode())
                if n < 0:
                    raise RuntimeError(f"axon_stop_nrt_profile rc={n}")
                print(f"profile: {n} file(s) written to {output_dir}")

        return _hook

    def set_axon_ntff_profile_hook(hook):
        state["hook"] = hook
        state["inited"] = True

    def get_axon_ntff_profile_hook():
        if not state["inited"]:
            state["hook"] = _default_hook()
            state["inited"] = True
        return state["hook"]

    mod.set_axon_ntff_profile_hook = set_axon_ntff_profile_hook
    mod.get_axon_ntff_profile_hook = get_axon_ntff_profile_hook
    sys.modules["antenv.axon_hooks"] = mod


def kernel(x, w_qkv, b_qkv, w_proj, b_proj, _trace=False, _trace_kwargs=None):
    if _trace:
        _install_ntff_hook_module()
    from concourse.bass_utils import run_bass_kernel_spmd

    x = np.asarray(x, dtype=np.float32)
    b, c, h, w = x.shape
    assert (b, c, h, w) == (B, C, 32, 32)

    weights, bqkc, resid_bias = _pack_weights(w_qkv, b_qkv, w_proj, b_proj)
    weights = {k: _bf16(v) for k, v in weights.items()}
    weights["bqkc"] = np.ascontiguousarray(bqkc, dtype=np.float32)
    nc = _get_nc()

    in_maps = []
    for core in range(N_CORES):
        m = dict(weights)
        xm = np.ascontiguousarray(x[core].reshape(C, L))
        m["x"] = _bf16(xm)
        m["xf"] = xm + resid_bias[:, None].astype(np.float32)
        in_maps.append(m)

    res = run_bass_kernel_spmd(
        nc,
        in_maps,
        list(range(N_CORES)),
        trace=_trace,
        **(_trace_kwargs or {}),
    )
    out = np.stack([res.results[core]["out"] for core in range(N_CORES)])
    if _trace:
        _CACHE["last_result"] = res
    return out.reshape(B, C, 32, 32)
